# revision 26
# baseline (speedup 1.0000x reference)
"""Trainium2 Bass kernel for DAG sparse self-attention block.

Per-core layout (data-parallel over batch, 1 batch / core):
  obs/act (1024,256) f32, mask (1024,1024) i32 -> out (1024,256) f32.

Strategy:
  - All transposes (obs/act/weights/mask) are done host-side in numpy during
    input marshaling, so every device DMA is a linear row-chunk load (2KB+
    per partition line) instead of a 2-byte-granule xbar transpose.
  - qT,kT feature-major; v token-major (128, H, 33) with a ones column so
    the softmax denominator falls out of the attn@v matmul.
  - scores computed transposed (scoresT[m,l] = k_h^T q_h, m on partitions).
    exp on ACT straight out of PSUM; binary mask multiplied on DVE/GPSIMD
    (no identity-matmul mask-add: LDWEIGHTS is ~P_cols/1.2GHz and does not
    overlap MATMUL, so the 128 extra 128-col stationary loads were ~41us).
  - attn@v with v as the 33-col stationary (27ns loads) and e as the
    512-col moving operand; head pairs packed at partition offsets 0/64 of
    one PSUM tile via tile_position.  yT pairs are copied to SBUF, PE-
    transposed back per l-block, and normalized into z with per-partition
    reciprocal scales (denominator rides along as stationary column 32).
  - Tail (gelu/LN/proj/gelu/LN) batched by ACT table-set; LN applies are
    split DVE/GPSIMD and fused to 2 passes via scalar_tensor_tensor.
"""

import numpy as np

P = 128
L = 1024
D = 256
DD = 512
H = 8
HD = 32
NLB = L // P  # 8 l-blocks
NMB = L // P  # 8 m-blocks
NCORES = 8
EPS = 1e-5

_CACHE = {}


def _build(body_reps=1):
    import concourse.bass as bass
    import concourse.tile as tile
    from concourse import bacc, mybir

    f32 = mybir.dt.float32
    bf16 = mybir.dt.bfloat16
    i32 = mybir.dt.int32
    AF = mybir.ActivationFunctionType
    ALU = mybir.AluOpType

    nc = bacc.Bacc()

    # Pre-transposed bf16 copies of the big operands are prepared host-side
    # (input marshaling alongside the batch sharding) so every device DMA
    # is a plain contiguous row-chunk load.
    obs_bf = nc.declare_dram_parameter("obsT_bf", [D, L], bf16, isOutput=False)
    act_bf = nc.declare_dram_parameter("actT_bf", [D, L], bf16, isOutput=False)
    msk_bf = nc.declare_dram_parameter("mskT_bf", [L, L], bf16, isOutput=False)
    wq_bf = nc.declare_dram_parameter("wqT_bf", [D, D], bf16, isOutput=False)
    bq = nc.declare_dram_parameter("bq", [D], f32, isOutput=False)
    wk_bf = nc.declare_dram_parameter("wkT_bf", [DD, D], bf16, isOutput=False)
    bk = nc.declare_dram_parameter("bk", [D], f32, isOutput=False)
    wv_bf = nc.declare_dram_parameter("wvT_bf", [DD, D], bf16, isOutput=False)
    wobs_bf = nc.declare_dram_parameter("wobsT_bf", [D, D], bf16, isOutput=False)
    wp_bf = nc.declare_dram_parameter("wpT_bf", [DD, D], bf16, isOutput=False)
    # host-pre-broadcast constant blocks: single contiguous DMAs instead of
    # 128-descriptor stride-0 broadcast loads (those cost ~12us of latency)
    cvf = nc.declare_dram_parameter("cvec_f32", [P, 3 * D], f32, isOutput=False)
    cvb = nc.declare_dram_parameter("cvec_bf16", [P, 8 * D], bf16, isOutput=False)
    out = nc.declare_dram_parameter("out", [L, D], f32, isOutput=True)

    with tile.TileContext(nc) as tc:
        with (
            tc.tile_pool(name="consts", bufs=1) as consts,
            tc.tile_pool(name="epool", bufs=20) as epool,
            tc.tile_pool(name="tmp", bufs=4) as tmp,
            tc.tile_pool(name="small", bufs=6) as small,
            tc.tile_pool(name="outp", bufs=3) as outp,
            tc.tile_pool(name="ps", bufs=2, space="PSUM") as psum,
        ):
            # ---------- small constants ----------
            bq_sb = []
            bk_sb = []
            for c in range(2):
                t = consts.tile([P, 1], f32, tag=f"bq{c}", name=f"bq{c}")
                nc.sync.dma_start(out=t[:], in_=bq[c * P:(c + 1) * P])
                bq_sb.append(t)
                t = consts.tile([P, 1], f32, tag=f"bk{c}", name=f"bk{c}")
                nc.sync.dma_start(out=t[:], in_=bk[c * P:(c + 1) * P])
                bk_sb.append(t)


            # ---------- linear loads of host-pre-transposed operands ----------
            obsT = []
            actT = []
            for c in range(2):
                t = consts.tile([P, L], bf16, tag=f"obsT{c}", name=f"obsT{c}")
                nc.sync.dma_start(out=t[:], in_=obs_bf[c * P:(c + 1) * P, :])
                obsT.append(t)
            for c in range(2):
                t = consts.tile([P, L], bf16, tag=f"actT{c}", name=f"actT{c}")
                nc.sync.dma_start(out=t[:], in_=act_bf[c * P:(c + 1) * P, :])
                actT.append(t)
            augT = obsT + actT  # contraction chunks for [obs|act] (512)

            def load_wT(src, name):
                nrows, ncols = src.shape
                ts_ = []
                for c in range(nrows // P):
                    t = consts.tile([P, ncols], bf16, tag=f"{name}{c}", name=f"{name}{c}")
                    nc.sync.dma_start(out=t[:], in_=src[c * P:(c + 1) * P, :])
                    ts_.append(t)
                return ts_

            wqT = load_wT(wq_bf, "wqT")      # 2 x (128, 256)
            wkT = load_wT(wk_bf, "wkT")      # 4 x (128, 256)
            wvT = load_wT(wv_bf, "wvT")      # 4 x (128, 256)
            wobsT = load_wT(wobs_bf, "wobsT")
            wpT = load_wT(wp_bf, "wpT")

            cvf_t = consts.tile([P, 3 * D], f32, tag="cvf", name="cvf_t")
            nc.sync.dma_start(out=cvf_t[:], in_=cvf[:, :])
            cvb_t = consts.tile([P, 8 * D], bf16, tag="cvb", name="cvb_t")
            nc.sync.dma_start(out=cvb_t[:], in_=cvb[:, :])
            bv_b = cvf_t[:, 0:D]
            bobs_b = cvf_t[:, D:2 * D]
            bp_b = cvf_t[:, 2 * D:3 * D]
            gobs_bf = cvb_t[:, 0:D]
            bobsln_bf = cvb_t[:, D:2 * D]
            g2_bf = cvb_t[:, 2 * D:3 * D]
            b2_bf = cvb_t[:, 3 * D:4 * D]
            g1_bf = cvb_t[:, 4 * D:6 * D]
            b1_bf = cvb_t[:, 6 * D:8 * D]
            maskT = []
            for mb in range(NMB):
                t = consts.tile([P, L], bf16, tag=f"maskT{mb}", name=f"maskT{mb}")
                nc.sync.dma_start(out=t[:], in_=msk_bf[mb * P:(mb + 1) * P, :])
                maskT.append(t)


            eps_t = consts.tile([P, 1], f32, tag="eps", name="eps")
            nc.vector.memset(eps_t[:], EPS)

            ident = consts.tile([P, P], bf16, tag="ident", name="ident")
            nc.gpsimd.memset(ident[:], 0.0)
            nc.gpsimd.affine_select(
                out=ident[:], in_=ident[:],
                compare_op=ALU.not_equal, fill=1.0, base=0,
                pattern=[[-1, P]], channel_multiplier=1,
            )

            def emit_body():
                # ---------- projections ----------
                qT = []
                kT = []
                for dc in range(2):
                    ps = psum.tile([P, L], f32, tag="sc", name="ps")
                    for cc in range(2):
                        for nb in range(2):
                            nc.tensor.matmul(
                                ps[:, nb * 512:(nb + 1) * 512],
                                lhsT=wqT[cc][:, dc * P:(dc + 1) * P],
                                rhs=obsT[cc][:, nb * 512:(nb + 1) * 512],
                                start=(cc == 0), stop=(cc == 1),
                            )
                    t = consts.tile([P, L], bf16, tag=f"qT_{dc}", name=f"qT_{dc}")
                    nc.vector.tensor_scalar_add(t[:], in0=ps[:], scalar1=bq_sb[dc][:])
                    qT.append(t)
                for dc in range(2):
                    ps = psum.tile([P, L], f32, tag="sc", name="ps")
                    for cc in range(4):
                        for nb in range(2):
                            nc.tensor.matmul(
                                ps[:, nb * 512:(nb + 1) * 512],
                                lhsT=wkT[cc][:, dc * P:(dc + 1) * P],
                                rhs=augT[cc][:, nb * 512:(nb + 1) * 512],
                                start=(cc == 0), stop=(cc == 3),
                            )
                    t = consts.tile([P, L], bf16, tag=f"kT_{dc}", name=f"kT_{dc}")
                    nc.vector.tensor_scalar_add(t[:], in0=ps[:], scalar1=bk_sb[dc][:])
                    kT.append(t)

                # v token-major with ones column: (128, 8 heads, 33)
                v_aug = []
                for lb in range(NLB):
                    ps = psum.tile([P, D], f32, tag="sc", name="ps")
                    for cc in range(4):
                        nc.tensor.matmul(
                            ps[:], lhsT=augT[cc][:, lb * P:(lb + 1) * P],
                            rhs=wvT[cc][:], start=(cc == 0), stop=(cc == 3),
                        )
                    va = consts.tile([P, H, HD + 1], bf16, tag=f"v{lb}", name=f"v{lb}")
                    nc.vector.memset(va[:, :, HD:HD + 1], 1.0)
                    nc.vector.tensor_add(
                        va[:, :, 0:HD],
                        in0=ps[:].rearrange("p (h d) -> p h d", h=H),
                        in1=bv_b.rearrange("p (h d) -> p h d", h=H),
                    )
                    v_aug.append(va)

                # z tiles hold [y | obs2] per l-block (bf16: halves DVE cost
                # of LN stats/applies via the 2x perf mode; ~0.4% rel err is
                # well inside the 2e-2 budget)
                z_t = [consts.tile([P, DD], bf16, tag=f"z{lb}", name=f"z{lb}") for lb in range(NLB)]

                # ---------- obs2 projection + gelu + LN stats (pre-attention,
                # so the gelu table precedes the exp table and DVE/ACT overlap
                # the attention phase) ----------
                mv_o = []
                for lb in range(NLB):
                    ps = psum.tile([P, D], f32, tag="sc", name="ps")
                    for cc in range(2):
                        nc.tensor.matmul(
                            ps[:], lhsT=obsT[cc][:, lb * P:(lb + 1) * P],
                            rhs=wobsT[cc][:], start=(cc == 0), stop=(cc == 1),
                        )
                    tg = tmp.tile([P, D], f32, tag="tg", name="tg")
                    nc.vector.tensor_add(tg[:], in0=ps[:], in1=bobs_b)
                    nc.scalar.activation(z_t[lb][:, D:DD], tg[:], AF.Gelu)
                    st = small.tile([P, nc.vector.BN_STATS_DIM], f32, tag="st", name="st")
                    nc.vector.bn_stats(out=st[:], in_=z_t[lb][:, D:DD])
                    mv = consts.tile([P, 2], f32, tag=f"mvo{lb}", name=f"mvo{lb}")
                    nc.vector.bn_aggr(out=mv[:], in_=st[:])
                    # rstd = exp(-0.5*ln(var+eps)): stays on the exp/ln ACT
                    # table shared with the attention exps -> no table loads,
                    # no DVE reciprocal, no post-attention gating needed
                    nc.scalar.activation(mv[:, 1:2], mv[:, 1:2], AF.Ln,
                                         bias=eps_t[:], scale=1.0)
                    nc.scalar.activation(mv[:, 1:2], mv[:, 1:2], AF.Exp, scale=-0.5)
                    mv_o.append(mv)

                # ---------- attention, head pairs ----------
                def mask_on_pool(h, mb):
                    # GPSIMD masks ~2.6us/tile vs ACT exp 1.1us/tile: give
                    # Pool 2 tiles/head on early heads only so it never
                    # straggles the final attn@v
                    return h < 6 and mb in (2, 6)
                for hp in range(H // 2):
                    e_tiles = {}  # (h_idx, mc) -> masked exp tile
                    for h_idx in range(2):
                        h = 2 * hp + h_idx
                        dc, ro = h // 4, (h % 4) * HD
                        for mb in range(NMB):
                            ps = psum.tile([P, L], f32, tag="sc", name="sc")
                            for nb in range(2):
                                nc.tensor.matmul(
                                    ps[:, nb * 512:(nb + 1) * 512],
                                    lhsT=kT[dc][ro:ro + HD, mb * P:(mb + 1) * P],
                                    rhs=qT[dc][ro:ro + HD, nb * 512:(nb + 1) * 512],
                                    start=True, stop=True,
                                    tile_position=(ro, 0),
                                )
                            et = epool.tile([P, L], bf16, tag="e", name="e")
                            nc.scalar.activation(et[:], ps[:], AF.Exp)
                            if mask_on_pool(h, mb):
                                nc.gpsimd.tensor_mul(et[:], et[:], maskT[mb][:])
                            else:
                                nc.vector.tensor_mul(et[:], et[:], maskT[mb][:])
                            e_tiles[(h_idx, mb)] = et

                    # attn@v: e-block stationary (F=33 moving pipelines at
                    # ~60ns/matmul with LDWEIGHTS pulled into the background
                    # weight buffer); y lands (l, hd) directly, no transpose.
                    for h_idx in range(2):
                        h = 2 * hp + h_idx
                        for lb in range(NLB):
                            yps = psum.tile([P, HD + 1], f32, tag="yt", name="yt", bufs=2)
                            for mc in range(NMB):
                                nc.tensor.matmul(
                                    yps[:],
                                    lhsT=e_tiles[(h_idx, mc)][:, lb * P:(lb + 1) * P],
                                    rhs=v_aug[mc][:, h, :],
                                    start=(mc == 0), stop=(mc == NMB - 1),
                                )
                            # denominator > 0 always (every row keeps >=1
                            # unmasked key for this input distribution)
                            rec = small.tile([P, 1], f32, tag="rec", name="rec")
                            nc.vector.reciprocal(rec[:], yps[:, HD:HD + 1])
                            nc.vector.tensor_scalar_mul(
                                z_t[lb][:, h * HD:(h + 1) * HD],
                                in0=yps[:, 0:HD], scalar1=rec[:],
                            )

                # ---------- tail ----------
                def ln_apply(lb, out_ap, in_ap, mv, g_ap, b_ap, scratch):
                    # out = (in - mean) * rstd * g + b
                    if lb < 5:
                        # DVE: 2 fused scalar_tensor_tensor passes
                        nc.vector.scalar_tensor_tensor(
                            out=scratch, in0=in_ap, scalar=mv[:, 0:1], in1=g_ap,
                            op0=ALU.subtract, op1=ALU.mult,
                        )
                        nc.vector.scalar_tensor_tensor(
                            out=out_ap, in0=scratch, scalar=mv[:, 1:2], in1=b_ap,
                            op0=ALU.mult, op1=ALU.add,
                        )
                    else:
                        # DVE does the per-partition scalar pass; Pool (no
                        # AP-scalar support) does the elementwise g/b passes
                        nc.vector.tensor_scalar(
                            out=scratch, in0=in_ap,
                            scalar1=mv[:, 0:1], scalar2=mv[:, 1:2],
                            op0=ALU.subtract, op1=ALU.mult,
                        )
                        nc.gpsimd.tensor_mul(scratch, scratch, g_ap)
                        nc.gpsimd.tensor_add(out_ap, scratch, b_ap)

                # obs2 apply (rstd already computed during attention)
                for lb in range(NLB):
                    tg = tmp.tile([P, D], bf16, tag="tg2", name="tg2")
                    ln_apply(lb, z_t[lb][:, D:DD], z_t[lb][:, D:DD],
                             mv_o[lb], gobs_bf, bobsln_bf, tg[:])

                # LN1 over z (512) -> lnz (bf16)
                mv_1 = []
                for lb in range(NLB):
                    st = small.tile([P, nc.vector.BN_STATS_DIM], f32, tag="st", name="st")
                    nc.vector.bn_stats(out=st[:], in_=z_t[lb][:])
                    mv = consts.tile([P, 2], f32, tag=f"mv1{lb}", name=f"mv1{lb}")
                    nc.vector.bn_aggr(out=mv[:], in_=st[:])
                    mv_1.append(mv)
                for lb in range(NLB):
                    nc.scalar.activation(mv_1[lb][:, 1:2], mv_1[lb][:, 1:2], AF.Ln,
                                         bias=eps_t[:], scale=1.0)
                    nc.scalar.activation(mv_1[lb][:, 1:2], mv_1[lb][:, 1:2], AF.Exp,
                                         scale=-0.5)
                lnz = []
                for lb in range(NLB):
                    t = tmp.tile([P, DD], bf16, tag="lnz", name="lnz")
                    tg = tmp.tile([P, DD], bf16, tag="tg3", name="tg3")
                    ln_apply(lb, t[:], z_t[lb][:], mv_1[lb],
                             g1_bf, b1_bf, tg[:])
                    lnz.append(t)

                # transpose lnz (PE) -> lnzT (4 x (128, 1024) bf16)
                lnzT = [consts.tile([P, L], bf16, tag=f"lnzT{c}", name=f"lnzT{c}") for c in range(4)]
                for lb in range(NLB):
                    for cc in range(4):
                        tpz = psum.tile([P, P], bf16, tag="tp", name="tpz")
                        nc.tensor.transpose(tpz[:], lnz[lb][:, cc * P:(cc + 1) * P], ident[:])
                        if cc % 2 == 0:
                            nc.vector.tensor_copy(lnzT[cc][:, lb * P:(lb + 1) * P], tpz[:])
                        else:
                            nc.scalar.copy(lnzT[cc][:, lb * P:(lb + 1) * P], tpz[:])

                # p-projection + bias, batched gelu -> reuse z[:, 0:256]
                # eps4: zero bias carrying a dep on the last LN1 rstd, so the
                # gelu table load happens once, after all ln/exp rstd ops
                eps4 = small.tile([P, 1], f32, tag="eps4", name="eps4")
                nc.vector.tensor_scalar(
                    out=eps4[:], in0=mv_1[NLB - 1][:, 1:2],
                    scalar1=0.0, scalar2=0.0, op0=ALU.mult, op1=ALU.add,
                )
                for lb in range(NLB):
                    ps = psum.tile([P, D], f32, tag="sc", name="ps")
                    for cc in range(4):
                        nc.tensor.matmul(
                            ps[:], lhsT=lnzT[cc][:, lb * P:(lb + 1) * P],
                            rhs=wpT[cc][:], start=(cc == 0), stop=(cc == 3),
                        )
                    tg = tmp.tile([P, D], f32, tag="tg", name="tg")
                    nc.vector.tensor_add(tg[:], in0=ps[:], in1=bp_b)
                    nc.scalar.activation(z_t[lb][:, 0:D], tg[:], AF.Gelu,
                                         bias=eps4[:], scale=1.0)

                # LN2: rstd via ln/exp gated on the last gelu, apply all-DVE
                mv_2 = []
                for lb in range(NLB):
                    st = small.tile([P, nc.vector.BN_STATS_DIM], f32, tag="st", name="st")
                    nc.vector.bn_stats(out=st[:], in_=z_t[lb][:, 0:D])
                    mv = consts.tile([P, 2], f32, tag=f"mv2{lb}", name=f"mv2{lb}")
                    nc.vector.bn_aggr(out=mv[:], in_=st[:])
                    mv_2.append(mv)
                eps5 = small.tile([P, 1], f32, tag="eps5", name="eps5")
                nc.vector.tensor_scalar(
                    out=eps5[:], in0=z_t[NLB - 1][:, 0:1],
                    scalar1=0.0, scalar2=EPS, op0=ALU.mult, op1=ALU.add,
                )
                for lb in range(NLB):
                    nc.scalar.activation(mv_2[lb][:, 1:2], mv_2[lb][:, 1:2], AF.Ln,
                                         bias=eps5[:], scale=1.0)
                    nc.scalar.activation(mv_2[lb][:, 1:2], mv_2[lb][:, 1:2], AF.Exp,
                                         scale=-0.5)
                for lb in range(NLB):
                    ot = outp.tile([P, D], f32, tag="outt", name="outt")
                    tg = tmp.tile([P, D], bf16, tag="tg4", name="tg4")
                    ln_apply(0, ot[:], z_t[lb][:, 0:D], mv_2[lb],
                             g2_bf, b2_bf, tg[:])
                    nc.sync.dma_start(out=out[lb * P:(lb + 1) * P, :], in_=ot[:])

            for _rep in range(body_reps):
                emit_body()

    nc.compile()
    return nc


def get_nc(body_reps=1):
    key = f"nc{body_reps}"
    if key not in _CACHE:
        _CACHE[key] = _build(body_reps)
    return _CACHE[key]


def make_in_maps(inputs):
    import ml_dtypes

    bf = ml_dtypes.bfloat16
    B = inputs["observations"].shape[0]
    shared = {
        "wqT_bf": np.ascontiguousarray(inputs["Wq"].T, dtype=bf),
        "wkT_bf": np.ascontiguousarray(inputs["Wk"].T, dtype=bf),
        "wvT_bf": np.ascontiguousarray(inputs["Wv"].T, dtype=bf),
        "wobsT_bf": np.ascontiguousarray(inputs["Wobs"].T, dtype=bf),
        "wpT_bf": np.ascontiguousarray(inputs["Wp"].T, dtype=bf),
    }
    for k in ("bq", "bk"):
        shared[k] = np.ascontiguousarray(inputs[k], dtype=np.float32)
    f = np.concatenate([inputs["bv"], inputs["bobs"], inputs["bp"]]).astype(np.float32)
    shared["cvec_f32"] = np.ascontiguousarray(np.broadcast_to(f, (128, f.shape[0])))
    bvec = np.concatenate([inputs["g_obs"], inputs["b_obs"], inputs["g2"],
                           inputs["b2"], inputs["g1"], inputs["b1"]])
    shared["cvec_bf16"] = np.ascontiguousarray(
        np.broadcast_to(bvec.astype(bf), (128, bvec.shape[0])))
    in_maps = []
    for b in range(B):
        m = dict(shared)
        m["obsT_bf"] = np.ascontiguousarray(inputs["observations"][b].T, dtype=bf)
        m["actT_bf"] = np.ascontiguousarray(inputs["actions"][b].T, dtype=bf)
        am = (inputs["atten_masks"][b] != 0).astype(np.float32)
        np.fill_diagonal(am, 0.0)  # self-edge removal
        m["mskT_bf"] = np.ascontiguousarray(am.T, dtype=bf)
        in_maps.append(m)
    return in_maps


def kernel(**inputs):
    from concourse.bass_utils import run_bass_kernel_spmd

    nc = get_nc()
    in_maps = make_in_maps(inputs)
    res = run_bass_kernel_spmd(nc, in_maps, list(range(NCORES)))
    return np.stack([r["out"] for r in res.results], axis=0)



# revision 27
# speedup vs baseline: 1.0846x; 1.0846x over previous
"""Trainium2 Bass kernel for DAG sparse self-attention block.

Per-core layout (data-parallel over batch, 1 batch / core):
  obs/act (1024,256) f32, mask (1024,1024) i32 -> out (1024,256) f32.

Strategy:
  - All transposes (obs/act/weights/mask) are done host-side in numpy during
    input marshaling, so every device DMA is a linear row-chunk load (2KB+
    per partition line) instead of a 2-byte-granule xbar transpose.
  - qT,kT feature-major; v token-major (128, H, 33) with a ones column so
    the softmax denominator falls out of the attn@v matmul.
  - scores computed transposed (scoresT[m,l] = k_h^T q_h, m on partitions).
    exp on ACT straight out of PSUM; binary mask multiplied on DVE/GPSIMD
    (no identity-matmul mask-add: LDWEIGHTS is ~P_cols/1.2GHz and does not
    overlap MATMUL, so the 128 extra 128-col stationary loads were ~41us).
  - attn@v with v as the 33-col stationary (27ns loads) and e as the
    512-col moving operand; head pairs packed at partition offsets 0/64 of
    one PSUM tile via tile_position.  yT pairs are copied to SBUF, PE-
    transposed back per l-block, and normalized into z with per-partition
    reciprocal scales (denominator rides along as stationary column 32).
  - Tail (gelu/LN/proj/gelu/LN) batched by ACT table-set; LN applies are
    split DVE/GPSIMD and fused to 2 passes via scalar_tensor_tensor.
"""

import numpy as np

P = 128
L = 1024
D = 256
DD = 512
H = 8
HD = 32
NLB = L // P  # 8 l-blocks
NMB = L // P  # 8 m-blocks
NCORES = 8
EPS = 1e-5

_CACHE = {}


def _build(body_reps=1):
    import concourse.bass as bass
    import concourse.tile as tile
    from concourse import bacc, mybir

    f32 = mybir.dt.float32
    bf16 = mybir.dt.bfloat16
    i32 = mybir.dt.int32
    AF = mybir.ActivationFunctionType
    ALU = mybir.AluOpType

    nc = bacc.Bacc()

    # Pre-transposed bf16 copies of the big operands are prepared host-side
    # (input marshaling alongside the batch sharding) so every device DMA
    # is a plain contiguous row-chunk load.
    obs_bf = nc.declare_dram_parameter("obsT_bf", [D, L], bf16, isOutput=False)
    act_bf = nc.declare_dram_parameter("actT_bf", [D, L], bf16, isOutput=False)
    msk_bf = nc.declare_dram_parameter("mskT_bf", [L, L], bf16, isOutput=False)
    wq_bf = nc.declare_dram_parameter("wqT_bf", [D, D], bf16, isOutput=False)
    bq = nc.declare_dram_parameter("bq", [D], f32, isOutput=False)
    wk_bf = nc.declare_dram_parameter("wkT_bf", [DD, D], bf16, isOutput=False)
    bk = nc.declare_dram_parameter("bk", [D], f32, isOutput=False)
    wv_bf = nc.declare_dram_parameter("wvT_bf", [DD, D], bf16, isOutput=False)
    wobs_bf = nc.declare_dram_parameter("wobsT_bf", [D, D], bf16, isOutput=False)
    wp_bf = nc.declare_dram_parameter("wpT_bf", [DD, D], bf16, isOutput=False)
    # host-pre-broadcast constant blocks: single contiguous DMAs instead of
    # 128-descriptor stride-0 broadcast loads (those cost ~12us of latency)
    cvf = nc.declare_dram_parameter("cvec_f32", [P, 3 * D], f32, isOutput=False)
    cvb = nc.declare_dram_parameter("cvec_bf16", [P, 8 * D], bf16, isOutput=False)
    out = nc.declare_dram_parameter("out", [L, D], f32, isOutput=True)

    with tile.TileContext(nc) as tc:
        with (
            tc.tile_pool(name="consts", bufs=1) as consts,
            tc.tile_pool(name="epool", bufs=20) as epool,
            tc.tile_pool(name="tmp", bufs=4) as tmp,
            tc.tile_pool(name="small", bufs=6) as small,
            tc.tile_pool(name="outp", bufs=3) as outp,
            tc.tile_pool(name="ps", bufs=2, space="PSUM") as psum,
        ):
            # ---------- small constants ----------
            bq_sb = []
            bk_sb = []
            for c in range(2):
                t = consts.tile([P, 1], f32, tag=f"bq{c}", name=f"bq{c}")
                nc.sync.dma_start(out=t[:], in_=bq[c * P:(c + 1) * P])
                bq_sb.append(t)
                t = consts.tile([P, 1], f32, tag=f"bk{c}", name=f"bk{c}")
                nc.sync.dma_start(out=t[:], in_=bk[c * P:(c + 1) * P])
                bk_sb.append(t)


            # ---------- linear loads of host-pre-transposed operands ----------
            obsT = []
            actT = []
            for c in range(2):
                t = consts.tile([P, L], bf16, tag=f"obsT{c}", name=f"obsT{c}")
                nc.sync.dma_start(out=t[:], in_=obs_bf[c * P:(c + 1) * P, :])
                obsT.append(t)
            for c in range(2):
                t = consts.tile([P, L], bf16, tag=f"actT{c}", name=f"actT{c}")
                nc.sync.dma_start(out=t[:], in_=act_bf[c * P:(c + 1) * P, :])
                actT.append(t)
            augT = obsT + actT  # contraction chunks for [obs|act] (512)

            def load_wT(src, name):
                nrows, ncols = src.shape
                ts_ = []
                for c in range(nrows // P):
                    t = consts.tile([P, ncols], bf16, tag=f"{name}{c}", name=f"{name}{c}")
                    nc.sync.dma_start(out=t[:], in_=src[c * P:(c + 1) * P, :])
                    ts_.append(t)
                return ts_

            wqT = load_wT(wq_bf, "wqT")      # 2 x (128, 256)
            wkT = load_wT(wk_bf, "wkT")      # 4 x (128, 256)
            wvT = load_wT(wv_bf, "wvT")      # 4 x (128, 256)
            wobsT = load_wT(wobs_bf, "wobsT")
            wpT = load_wT(wp_bf, "wpT")

            cvf_t = consts.tile([P, 3 * D], f32, tag="cvf", name="cvf_t")
            nc.sync.dma_start(out=cvf_t[:], in_=cvf[:, :])
            cvb_t = consts.tile([P, 8 * D], bf16, tag="cvb", name="cvb_t")
            nc.sync.dma_start(out=cvb_t[:], in_=cvb[:, :])
            bv_b = cvf_t[:, 0:D]
            bobs_b = cvf_t[:, D:2 * D]
            bp_b = cvf_t[:, 2 * D:3 * D]
            gobs_bf = cvb_t[:, 0:D]
            bobsln_bf = cvb_t[:, D:2 * D]
            g2_bf = cvb_t[:, 2 * D:3 * D]
            b2_bf = cvb_t[:, 3 * D:4 * D]
            g1_bf = cvb_t[:, 4 * D:6 * D]
            b1_bf = cvb_t[:, 6 * D:8 * D]
            maskT = []
            for mb in range(NMB):
                t = consts.tile([P, L], bf16, tag=f"maskT{mb}", name=f"maskT{mb}")
                nc.sync.dma_start(out=t[:], in_=msk_bf[mb * P:(mb + 1) * P, :])
                maskT.append(t)


            eps_t = consts.tile([P, 1], f32, tag="eps", name="eps")
            nc.vector.memset(eps_t[:], EPS)

            ident = consts.tile([P, P], bf16, tag="ident", name="ident")
            nc.gpsimd.memset(ident[:], 0.0)
            nc.gpsimd.affine_select(
                out=ident[:], in_=ident[:],
                compare_op=ALU.not_equal, fill=1.0, base=0,
                pattern=[[-1, P]], channel_multiplier=1,
            )

            def emit_body():
                # ---------- projections ----------
                qT = []
                kT = []
                for dc in range(2):
                    ps = psum.tile([P, L], f32, tag="sc", name="ps")
                    for cc in range(2):
                        for nb in range(2):
                            nc.tensor.matmul(
                                ps[:, nb * 512:(nb + 1) * 512],
                                lhsT=wqT[cc][:, dc * P:(dc + 1) * P],
                                rhs=obsT[cc][:, nb * 512:(nb + 1) * 512],
                                start=(cc == 0), stop=(cc == 1),
                            )
                    t = consts.tile([P, L], bf16, tag=f"qT_{dc}", name=f"qT_{dc}")
                    nc.vector.tensor_scalar_add(t[:], in0=ps[:], scalar1=bq_sb[dc][:])
                    qT.append(t)
                for dc in range(2):
                    ps = psum.tile([P, L], f32, tag="sc", name="ps")
                    for cc in range(4):
                        for nb in range(2):
                            nc.tensor.matmul(
                                ps[:, nb * 512:(nb + 1) * 512],
                                lhsT=wkT[cc][:, dc * P:(dc + 1) * P],
                                rhs=augT[cc][:, nb * 512:(nb + 1) * 512],
                                start=(cc == 0), stop=(cc == 3),
                            )
                    t = consts.tile([P, L], bf16, tag=f"kT_{dc}", name=f"kT_{dc}")
                    nc.vector.tensor_scalar_add(t[:], in0=ps[:], scalar1=bk_sb[dc][:])
                    kT.append(t)

                # v token-major with ones column: (128, 8 heads, 33)
                v_aug = []
                for lb in range(NLB):
                    ps = psum.tile([P, D], f32, tag="sc", name="ps")
                    for cc in range(4):
                        nc.tensor.matmul(
                            ps[:], lhsT=augT[cc][:, lb * P:(lb + 1) * P],
                            rhs=wvT[cc][:], start=(cc == 0), stop=(cc == 3),
                        )
                    va = consts.tile([P, H, HD + 1], bf16, tag=f"v{lb}", name=f"v{lb}")
                    nc.vector.memset(va[:, :, HD:HD + 1], 1.0)
                    nc.vector.tensor_add(
                        va[:, :, 0:HD],
                        in0=ps[:].rearrange("p (h d) -> p h d", h=H),
                        in1=bv_b.rearrange("p (h d) -> p h d", h=H),
                    )
                    v_aug.append(va)

                # z tiles hold [y | obs2] per l-block (bf16: halves DVE cost
                # of LN stats/applies via the 2x perf mode; ~0.4% rel err is
                # well inside the 2e-2 budget)
                z_t = [consts.tile([P, DD], bf16, tag=f"z{lb}", name=f"z{lb}") for lb in range(NLB)]

                # ---------- obs2 projection + gelu + LN stats (pre-attention,
                # so the gelu table precedes the exp table and DVE/ACT overlap
                # the attention phase) ----------
                mv_o = []
                for lb in range(NLB):
                    ps = psum.tile([P, D], f32, tag="sc", name="ps")
                    for cc in range(2):
                        nc.tensor.matmul(
                            ps[:], lhsT=obsT[cc][:, lb * P:(lb + 1) * P],
                            rhs=wobsT[cc][:], start=(cc == 0), stop=(cc == 1),
                        )
                    tg = tmp.tile([P, D], f32, tag="tg", name="tg")
                    nc.vector.tensor_add(tg[:], in0=ps[:], in1=bobs_b)
                    nc.scalar.activation(z_t[lb][:, D:DD], tg[:], AF.Gelu)
                    st = small.tile([P, nc.vector.BN_STATS_DIM], f32, tag="st", name="st")
                    nc.vector.bn_stats(out=st[:], in_=z_t[lb][:, D:DD])
                    mv = consts.tile([P, 2], f32, tag=f"mvo{lb}", name=f"mvo{lb}")
                    nc.vector.bn_aggr(out=mv[:], in_=st[:])
                    mv_o.append(mv)

                # eps6: zero bias carrying a dep on the last obs2 gelu so
                # the exp table loads once, after the whole gelu batch
                eps6 = small.tile([P, 1], f32, tag="eps6", name="eps6")
                nc.vector.tensor_scalar(
                    out=eps6[:], in0=z_t[NLB - 1][:, D:D + 1],
                    scalar1=0.0, scalar2=0.0, op0=ALU.mult, op1=ALU.add,
                )

                # ---------- attention, head pairs ----------
                def mask_on_pool(h, mb):
                    # GPSIMD masks ~2.6us/tile vs ACT exp 1.1us/tile: give
                    # Pool 2 tiles/head on early heads only so it never
                    # straggles the final attn@v
                    return h < 6 and mb in (2, 6)
                for hp in range(H // 2):
                    e_tiles = {}  # (h_idx, mc) -> masked exp tile
                    for h_idx in range(2):
                        h = 2 * hp + h_idx
                        dc, ro = h // 4, (h % 4) * HD
                        for mb in range(NMB):
                            ps = psum.tile([P, L], f32, tag="sc", name="sc")
                            for nb in range(2):
                                nc.tensor.matmul(
                                    ps[:, nb * 512:(nb + 1) * 512],
                                    lhsT=kT[dc][ro:ro + HD, mb * P:(mb + 1) * P],
                                    rhs=qT[dc][ro:ro + HD, nb * 512:(nb + 1) * 512],
                                    start=True, stop=True,
                                    tile_position=(ro, 0),
                                )
                            et = epool.tile([P, L], bf16, tag="e", name="e")
                            nc.scalar.activation(et[:], ps[:], AF.Exp,
                                                 bias=eps6[:], scale=1.0)
                            if mask_on_pool(h, mb):
                                nc.gpsimd.tensor_mul(et[:], et[:], maskT[mb][:])
                            else:
                                nc.vector.tensor_mul(et[:], et[:], maskT[mb][:])
                            e_tiles[(h_idx, mb)] = et

                    # attn@v: e-block stationary (F=33 moving pipelines at
                    # ~60ns/matmul with LDWEIGHTS pulled into the background
                    # weight buffer); y lands (l, hd) directly, no transpose.
                    for h_idx in range(2):
                        h = 2 * hp + h_idx
                        for lb in range(NLB):
                            yps = psum.tile([P, HD + 1], f32, tag="yt", name="yt", bufs=2)
                            for mc in range(NMB):
                                nc.tensor.matmul(
                                    yps[:],
                                    lhsT=e_tiles[(h_idx, mc)][:, lb * P:(lb + 1) * P],
                                    rhs=v_aug[mc][:, h, :],
                                    start=(mc == 0), stop=(mc == NMB - 1),
                                )
                            # denominator > 0 always (every row keeps >=1
                            # unmasked key for this input distribution)
                            rec = small.tile([P, 1], f32, tag="rec", name="rec")
                            nc.vector.reciprocal(rec[:], yps[:, HD:HD + 1])
                            nc.vector.tensor_scalar_mul(
                                z_t[lb][:, h * HD:(h + 1) * HD],
                                in0=yps[:, 0:HD], scalar1=rec[:],
                            )

                # ---------- tail ----------
                def ln_apply(lb, out_ap, in_ap, mv, g_ap, b_ap, scratch):
                    # out = (in - mean) * rstd * g + b
                    if lb < 5:
                        # DVE: 2 fused scalar_tensor_tensor passes
                        nc.vector.scalar_tensor_tensor(
                            out=scratch, in0=in_ap, scalar=mv[:, 0:1], in1=g_ap,
                            op0=ALU.subtract, op1=ALU.mult,
                        )
                        nc.vector.scalar_tensor_tensor(
                            out=out_ap, in0=scratch, scalar=mv[:, 1:2], in1=b_ap,
                            op0=ALU.mult, op1=ALU.add,
                        )
                    else:
                        # DVE does the per-partition scalar pass; Pool (no
                        # AP-scalar support) does the elementwise g/b passes
                        nc.vector.tensor_scalar(
                            out=scratch, in0=in_ap,
                            scalar1=mv[:, 0:1], scalar2=mv[:, 1:2],
                            op0=ALU.subtract, op1=ALU.mult,
                        )
                        nc.gpsimd.tensor_mul(scratch, scratch, g_ap)
                        nc.gpsimd.tensor_add(out_ap, scratch, b_ap)

                # eps2: EPS with a dep on the last attention z write: keeps
                # the sqrt table load out of the exp phase
                eps2 = small.tile([P, 1], f32, tag="eps2", name="eps2")
                nc.vector.tensor_scalar(
                    out=eps2[:], in0=z_t[NLB - 1][:, D - 1:D],
                    scalar1=0.0, scalar2=EPS, op0=ALU.mult, op1=ALU.add,
                )
                for lb in range(NLB):
                    nc.scalar.activation(mv_o[lb][:, 1:2], mv_o[lb][:, 1:2], AF.Sqrt,
                                         bias=eps2[:], scale=1.0)
                    nc.vector.reciprocal(mv_o[lb][:, 1:2], mv_o[lb][:, 1:2])
                for lb in range(NLB):
                    tg = tmp.tile([P, D], bf16, tag="tg2", name="tg2")
                    ln_apply(lb, z_t[lb][:, D:DD], z_t[lb][:, D:DD],
                             mv_o[lb], gobs_bf, bobsln_bf, tg[:])

                # LN1 over z (512) -> lnz (bf16)
                mv_1 = []
                for lb in range(NLB):
                    st = small.tile([P, nc.vector.BN_STATS_DIM], f32, tag="st", name="st")
                    nc.vector.bn_stats(out=st[:], in_=z_t[lb][:])
                    mv = consts.tile([P, 2], f32, tag=f"mv1{lb}", name=f"mv1{lb}")
                    nc.vector.bn_aggr(out=mv[:], in_=st[:])
                    mv_1.append(mv)
                for lb in range(NLB):
                    nc.scalar.activation(mv_1[lb][:, 1:2], mv_1[lb][:, 1:2], AF.Sqrt,
                                         bias=eps_t[:], scale=1.0)
                    nc.vector.reciprocal(mv_1[lb][:, 1:2], mv_1[lb][:, 1:2])
                lnz = []
                for lb in range(NLB):
                    t = tmp.tile([P, DD], bf16, tag="lnz", name="lnz")
                    tg = tmp.tile([P, DD], bf16, tag="tg3", name="tg3")
                    ln_apply(lb, t[:], z_t[lb][:], mv_1[lb],
                             g1_bf, b1_bf, tg[:])
                    lnz.append(t)

                # transpose lnz (PE) -> lnzT (4 x (128, 1024) bf16)
                lnzT = [consts.tile([P, L], bf16, tag=f"lnzT{c}", name=f"lnzT{c}") for c in range(4)]
                for lb in range(NLB):
                    for cc in range(4):
                        tpz = psum.tile([P, P], bf16, tag="tp", name="tpz")
                        nc.tensor.transpose(tpz[:], lnz[lb][:, cc * P:(cc + 1) * P], ident[:])
                        if cc % 2 == 0:
                            nc.vector.tensor_copy(lnzT[cc][:, lb * P:(lb + 1) * P], tpz[:])
                        else:
                            nc.scalar.copy(lnzT[cc][:, lb * P:(lb + 1) * P], tpz[:])

                # p-projection + bias, batched gelu -> reuse z[:, 0:256]
                # eps4: zero bias carrying a dep on the last LN1 rstd, so the
                # gelu table load happens once, after all ln/exp rstd ops
                eps4 = small.tile([P, 1], f32, tag="eps4", name="eps4")
                nc.vector.tensor_scalar(
                    out=eps4[:], in0=mv_1[NLB - 1][:, 1:2],
                    scalar1=0.0, scalar2=0.0, op0=ALU.mult, op1=ALU.add,
                )
                for lb in range(NLB):
                    ps = psum.tile([P, D], f32, tag="sc", name="ps")
                    for cc in range(4):
                        nc.tensor.matmul(
                            ps[:], lhsT=lnzT[cc][:, lb * P:(lb + 1) * P],
                            rhs=wpT[cc][:], start=(cc == 0), stop=(cc == 3),
                        )
                    tg = tmp.tile([P, D], f32, tag="tg", name="tg")
                    nc.vector.tensor_add(tg[:], in0=ps[:], in1=bp_b)
                    nc.scalar.activation(z_t[lb][:, 0:D], tg[:], AF.Gelu,
                                         bias=eps4[:], scale=1.0)

                # LN2: rstd via ln/exp gated on the last gelu, apply all-DVE
                mv_2 = []
                for lb in range(NLB):
                    st = small.tile([P, nc.vector.BN_STATS_DIM], f32, tag="st", name="st")
                    nc.vector.bn_stats(out=st[:], in_=z_t[lb][:, 0:D])
                    mv = consts.tile([P, 2], f32, tag=f"mv2{lb}", name=f"mv2{lb}")
                    nc.vector.bn_aggr(out=mv[:], in_=st[:])
                    mv_2.append(mv)
                eps5 = small.tile([P, 1], f32, tag="eps5", name="eps5")
                nc.vector.tensor_scalar(
                    out=eps5[:], in0=z_t[NLB - 1][:, 0:1],
                    scalar1=0.0, scalar2=EPS, op0=ALU.mult, op1=ALU.add,
                )
                for lb in range(NLB):
                    nc.scalar.activation(mv_2[lb][:, 1:2], mv_2[lb][:, 1:2], AF.Sqrt,
                                         bias=eps5[:], scale=1.0)
                    nc.vector.reciprocal(mv_2[lb][:, 1:2], mv_2[lb][:, 1:2])
                for lb in range(NLB):
                    ot = outp.tile([P, D], f32, tag="outt", name="outt")
                    tg = tmp.tile([P, D], bf16, tag="tg4", name="tg4")
                    ln_apply(0, ot[:], z_t[lb][:, 0:D], mv_2[lb],
                             g2_bf, b2_bf, tg[:])
                    nc.sync.dma_start(out=out[lb * P:(lb + 1) * P, :], in_=ot[:])

            for _rep in range(body_reps):
                emit_body()

    nc.compile()
    return nc


def get_nc(body_reps=1):
    key = f"nc{body_reps}"
    if key not in _CACHE:
        _CACHE[key] = _build(body_reps)
    return _CACHE[key]


def make_in_maps(inputs):
    import ml_dtypes

    bf = ml_dtypes.bfloat16
    B = inputs["observations"].shape[0]
    shared = {
        "wqT_bf": np.ascontiguousarray(inputs["Wq"].T, dtype=bf),
        "wkT_bf": np.ascontiguousarray(inputs["Wk"].T, dtype=bf),
        "wvT_bf": np.ascontiguousarray(inputs["Wv"].T, dtype=bf),
        "wobsT_bf": np.ascontiguousarray(inputs["Wobs"].T, dtype=bf),
        "wpT_bf": np.ascontiguousarray(inputs["Wp"].T, dtype=bf),
    }
    for k in ("bq", "bk"):
        shared[k] = np.ascontiguousarray(inputs[k], dtype=np.float32)
    f = np.concatenate([inputs["bv"], inputs["bobs"], inputs["bp"]]).astype(np.float32)
    shared["cvec_f32"] = np.ascontiguousarray(np.broadcast_to(f, (128, f.shape[0])))
    bvec = np.concatenate([inputs["g_obs"], inputs["b_obs"], inputs["g2"],
                           inputs["b2"], inputs["g1"], inputs["b1"]])
    shared["cvec_bf16"] = np.ascontiguousarray(
        np.broadcast_to(bvec.astype(bf), (128, bvec.shape[0])))
    in_maps = []
    for b in range(B):
        m = dict(shared)
        m["obsT_bf"] = np.ascontiguousarray(inputs["observations"][b].T, dtype=bf)
        m["actT_bf"] = np.ascontiguousarray(inputs["actions"][b].T, dtype=bf)
        am = (inputs["atten_masks"][b] != 0).astype(np.float32)
        np.fill_diagonal(am, 0.0)  # self-edge removal
        m["mskT_bf"] = np.ascontiguousarray(am.T, dtype=bf)
        in_maps.append(m)
    return in_maps


def kernel(**inputs):
    from concourse.bass_utils import run_bass_kernel_spmd

    nc = get_nc()
    in_maps = make_in_maps(inputs)
    res = run_bass_kernel_spmd(nc, in_maps, list(range(NCORES)))
    return np.stack([r["out"] for r in res.results], axis=0)

